# revision 39
# baseline (speedup 1.0000x reference)
"""Trainium2 Bass kernel for nn_FeatureGenKerasV2.

Contract: kernel(x) with x [100000, 115, 3] f32 -> [1, 200, 1198] f32.

Reference semantics:
  - global: cond = (count_nonzero(x[:,40:61]) > count_nonzero(x[:,94:115]))
  - per frame t<200: features built from hand(sel by cond)/pose/lip coords,
    temporal diff vs frame t+1, static-pair distances, hand mask.

Sharding (8 cores):
  - count phase: core c receives the two dense hand-column slices
    xl/xr = x[12500c:12500(c+1), 40:61|94:115, :] (column sharding of x),
    streams them at line rate and counts nonzeros (left on DVE, right on
    GpSimd), emitting per-partition partial counts.
  - feature phase: core c computes BOTH left/right feature variants for its
    output frames [25c, 25c+26) and writes yl_c/yr_c [25, 1198].
  - unshard: the host sums the per-core integer-valued partials, picks the
    variant (cond = cntL > cntR), and concatenates the per-core slices.
"""

import numpy as np

import concourse.bass as bass
import concourse.tile as tile
from concourse import bacc, mybir
from concourse import bass_utils

F32 = mybir.dt.float32
BF16 = mybir.dt.bfloat16
ALU = mybir.AluOpType

NCORES = 8
T_TOT = 100000
SHARD = T_TOT // NCORES          # 12500 count frames per core
P = 128                          # SBUF partitions used for counting; the
                                 # shard is zero-padded to 128*98 frames so
                                 # count DMAs have 128 descriptors (= 16
                                 # SDMA engines; 125 descs land on only 5)
FPP = 98                         # frames per partition (padded)
SHARD_PAD = P * FPP              # 12544
HW_ = FPP * 63                   # whole-hand row width per partition (6174)
CW = HW_ // 2                    # count chunk width (3087)
SW = CW // 2                     # tail sub-op width
OUTF = 25                        # output frames per core
BF = OUTF + 1                    # feature frames per core (1 halo)

# static pair index tables (match np.triu_indices order used by reference)
_HIU = np.triu_indices(21, 1)    # 210 hand pairs
_PIU = np.triu_indices(25, 1)    # 300 pose pairs
_LIU = np.triu_indices(20, 1)    # 190 lip pairs
NH, NP_, NL = 210, 300, 190

# packed constant layout (cst [65, 1406]):
#   [0:420)    GH  block-diag: rows 0:21 handL pairs, rows 21:42 handR pairs
#   [420:910)  G1  block-diag: rows 0:25 pose pairs, rows 25:45 outer-lip
#   [910:1100) G2  rows 45:65 inner-lip pairs
#   [1100:1406) M  rows 0:26: +-1 x-mirror mask over feat cols 0:306
CST_W = 1406
# packed per-core frame stacks (xstk [65, 130]):
#   [0:78)   hands,  rows 0:21 L / 21:42 R, col = coord*26 + frame
#   [78:130) shared, rows 0:25 pose / 25:45 lip1 / 45:65 lip2, 2 coords
XSTK_W = 130


def _pairs(g, iu, r0, c0):
    n = len(iu[0])
    g[r0 + iu[0], c0 + np.arange(n)] = 1.0
    g[r0 + iu[1], c0 + np.arange(n)] -= 1.0


def _build_cst():
    cst = np.zeros((65, CST_W), np.float32)
    _pairs(cst, _HIU, 0, 0)
    _pairs(cst, _HIU, 21, 210)
    _pairs(cst, _PIU, 0, 420)
    _pairs(cst, _LIU, 25, 720)
    _pairs(cst, _LIU, 45, 910)
    m = np.ones((26, 306), np.float32)
    m[:, 0:63:3] = -1.0      # hand x
    m[:, 63:153:2] = -1.0    # pose/lip x
    m[:, 153:216:3] = -1.0   # dhand x
    m[:, 216:306:2] = -1.0   # dpose/dlip x
    cst[0:26, 1100:1406] = m
    return cst


def _build_xstk(xb):
    # xb: [26, 115, 3]
    xstk = np.zeros((65, XSTK_W), np.float32)
    xstk[0:21, 0:78] = xb[:, 40:61, :].transpose(1, 2, 0).reshape(21, 78)
    xstk[21:42, 0:78] = xb[:, 94:115, :].transpose(1, 2, 0).reshape(21, 78)
    xstk[0:25, 78:130] = xb[:, 61:86, 0:2].transpose(1, 2, 0).reshape(25, 52)
    xstk[25:45, 78:130] = xb[:, 0:20, 0:2].transpose(1, 2, 0).reshape(20, 52)
    xstk[45:65, 78:130] = xb[:, 20:40, 0:2].transpose(1, 2, 0).reshape(20, 52)
    return xstk


def build_bass():
    nc = bacc.Bacc("TRN2", target_bir_lowering=False, debug=False,
                   num_devices=NCORES)

    xl = nc.dram_tensor("xl", [SHARD_PAD, 63], F32, kind="ExternalInput")
    xr = nc.dram_tensor("xr", [SHARD_PAD, 63], F32, kind="ExternalInput")
    xb = nc.dram_tensor("xb", [BF, 115, 3], F32, kind="ExternalInput")
    xstk = nc.dram_tensor("xstk", [65, XSTK_W], F32, kind="ExternalInput")
    cst = nc.dram_tensor("cst", [65, CST_W], F32, kind="ExternalInput")
    yl = nc.dram_tensor("yl", [OUTF, 1198], F32, kind="ExternalOutput")
    yr = nc.dram_tensor("yr", [OUTF, 1198], F32, kind="ExternalOutput")
    pdif = nc.dram_tensor("pdif", [P, 2], F32, kind="ExternalOutput")

    xlr = xl[:].rearrange("(p f) c -> p (f c)", p=P)   # [125, 6300]
    xrr = xr[:].rearrange("(p f) c -> p (f c)", p=P)

    with tile.TileContext(nc) as tc:
        with (
            tc.tile_pool(name="cl", bufs=2) as cl,
            tc.tile_pool(name="cr", bufs=2) as cr,
            tc.tile_pool(name="scr", bufs=2) as scr,
            tc.tile_pool(name="persist", bufs=1) as persist,
            tc.tile_pool(name="fb", bufs=1) as fb,
            tc.tile_pool(name="psum", bufs=1, space=bass.MemorySpace.PSUM) as psum,
        ):
            # ---------------- count phase ----------------
            # Two SWDGE chunks per hand (12.6KB descriptors, 16-engine
            # spread), streamed R0 L0 R1 L1; all counting is fused
            # not_equal+accumulate on DVE (CACHE_REDUCE). The final chunk's
            # compute is split into two column sub-ops to shorten the tail.
            accL = persist.tile([P, 3], F32)
            accR = persist.tile([P, 2], F32)
            # Stream order tuned so the DVE count chain ends right after the
            # last (small) chunk lands: R0 | feature loads | L0 R1 L1a L1b.
            tR0 = cr.tile([P, 3087], F32, tag="cR0", name="tR0")
            nc.sync.dma_start(tR0[:], xrr[:, 0:3087])

            # feature inputs next on the same FIFO ring: small (~0.5MB),
            # ready early, and never queued behind the bulk of the stream
            CST = fb.tile([65, CST_W], F32)
            nc.sync.dma_start(CST[:], cst[:])
            XSTK = fb.tile([65, XSTK_W], F32)
            nc.sync.dma_start(XSTK[:], xstk[:])
            XB = fb.tile([BF, 115, 3], F32)
            nc.sync.dma_start(XB[:], xb[:])
            XBs = fb.tile([OUTF, 115, 3], F32)
            nc.sync.dma_start(XBs[:], XB[1:BF, :, :])

            tL0 = cl.tile([P, 4116], F32, tag="cL0", name="tL0")
            nc.sync.dma_start(tL0[:], xlr[:, 0:4116])
            tR1 = cr.tile([P, 3087], F32, tag="cR1", name="tR1")
            nc.sync.dma_start(tR1[:], xrr[:, 3087:6174])
            tL1a = cl.tile([P, 1029], F32, tag="cL1a", name="tL1a")
            nc.sync.dma_start(tL1a[:], xlr[:, 4116:5145])
            tL1b = cl.tile([P, 1029], F32, tag="cL1b", name="tL1b")
            nc.sync.dma_start(tL1b[:], xlr[:, 5145:6174])

            def cache_count(src_ap, acc_col, w):
                s_ = scr.tile([P, w], BF16, tag=f"s{w}", name="s_")
                nc.vector.tensor_scalar(
                    out=s_[:], in0=src_ap, scalar1=0.0, scalar2=None,
                    op0=ALU.not_equal, op1=ALU.add, accum_out=acc_col)

            cache_count(tR0[:], accR[:, 0:1], 3087)
            cache_count(tL0[:], accL[:, 0:1], 4116)
            cache_count(tR1[:], accR[:, 1:2], 3087)
            cache_count(tL1a[:], accL[:, 1:2], 1029)
            cache_count(tL1b[:], accL[:, 2:3], 1029)

            red = persist.tile([P, 2], F32)
            nc.vector.reduce_sum(out=red[:, 0:1], in_=accL[:],
                                 axis=mybir.AxisListType.X)
            nc.vector.reduce_sum(out=red[:, 1:2], in_=accR[:],
                                 axis=mybir.AxisListType.X)
            nc.scalar.dma_start(pdif[:], red[:])

            # ---------------- feature phase ----------------
            D = fb.tile([OUTF, 115, 3], F32)
            nc.gpsimd.tensor_sub(D[:], XB[0:OUTF, :, :], XBs[:])

            # pairwise coordinate diffs via 7 block-diagonal matmuls
            ps = []
            for c in range(3):          # hands, coords x/y/z -> [26, 420]
                ph = psum.tile([BF, 420], F32, tag=f"ph{c}", name="ph")
                nc.tensor.matmul(ph[:], XSTK[0:42, c * BF:(c + 1) * BF],
                                 CST[0:42, 0:420])
                ps.append(ph)
            pa = []
            for c in range(2):          # pose+outer lip -> [26, 490]
                p_ = psum.tile([BF, 490], F32, tag=f"pa{c}", name="pa")
                nc.tensor.matmul(p_[:], XSTK[0:45, 78 + c * BF:78 + (c + 1) * BF],
                                 CST[0:45, 420:910])
                pa.append(p_)
            pb = []
            for c in range(2):          # inner lip -> [26, 190]
                p_ = psum.tile([BF, 190], F32, tag=f"pb{c}", name="pb")
                nc.tensor.matmul(p_[:], XSTK[0:65, 78 + c * BF:78 + (c + 1) * BF],
                                 CST[0:65, 910:1100])
                pb.append(p_)

            # dsq layout: [hL 210 | hR 210 | pose 300 | olip 190 | ilip 190]
            dsq = fb.tile([BF, 1100], F32)
            sy = fb.tile([BF, 1100], F32)
            sz = fb.tile([BF, 420], F32)
            nc.scalar.square(dsq[:, 0:420], ps[0][:])
            nc.scalar.square(sy[:, 0:420], ps[1][:])
            nc.scalar.square(sz[:], ps[2][:])
            nc.scalar.square(dsq[:, 420:910], pa[0][:])
            nc.scalar.square(sy[:, 420:910], pa[1][:])
            nc.scalar.square(dsq[:, 910:1100], pb[0][:])
            nc.scalar.square(sy[:, 910:1100], pb[1][:])
            nc.gpsimd.tensor_add(dsq[:], dsq[:], sy[:])
            nc.gpsimd.tensor_add(dsq[:, 0:420], dsq[:, 0:420], sz[:])

            FEATL = fb.tile([OUTF, 1198], F32)
            FEATR = fb.tile([OUTF, 1198], F32)

            def v3(ft, lo, hi):
                return ft[:, lo:hi].rearrange("p (j c) -> p j c", c=3)

            def v2(ft, lo, hi):
                return ft[:, lo:hi].rearrange("p (j c) -> p j c", c=2)

            # right variant coordinate blocks
            nc.scalar.copy(v3(FEATR, 0, 63), XB[0:OUTF, 94:115, :])
            nc.scalar.copy(v2(FEATR, 63, 113), XB[0:OUTF, 61:86, 0:2])
            nc.scalar.copy(v2(FEATR, 113, 153), XB[0:OUTF, 0:20, 0:2])
            nc.scalar.copy(v3(FEATR, 153, 216), D[:, 94:115, :])
            nc.scalar.copy(v2(FEATR, 216, 266), D[:, 61:86, 0:2])
            nc.scalar.copy(v2(FEATR, 266, 306), D[:, 0:20, 0:2])
            # left variant: own hand blocks, shared pose/lip copied from R,
            # then a single +-1 mirror multiply over cols 0:306
            nc.scalar.copy(v3(FEATL, 0, 63), XB[0:OUTF, 40:61, :])
            nc.scalar.copy(v3(FEATL, 153, 216), D[:, 40:61, :])
            nc.scalar.copy(FEATL[:, 63:153], FEATR[:, 63:153])
            nc.scalar.copy(FEATL[:, 216:306], FEATR[:, 216:306])
            nc.gpsimd.tensor_mul(FEATL[:, 0:306], FEATL[:, 0:306],
                                 CST[0:OUTF, 1100:1406])

            # distances
            nc.scalar.sqrt(FEATR[:, 306:1196], dsq[0:OUTF, 210:1100])
            nc.scalar.sqrt(FEATL[:, 306:516], dsq[0:OUTF, 0:210])
            nc.gpsimd.tensor_copy(FEATL[:, 516:1196], FEATR[:, 516:1196])

            # hand masks + token type ids
            sumL = fb.tile([BF, 1], F32)
            nc.vector.reduce_sum(out=sumL[:], in_=XB[:, 40:61, :],
                                 axis=mybir.AxisListType.XY)
            sumR = fb.tile([BF, 1], F32)
            nc.vector.reduce_sum(out=sumR[:], in_=XB[:, 94:115, :],
                                 axis=mybir.AxisListType.XY)
            for FT, sm in ((FEATL, sumL), (FEATR, sumR)):
                nc.vector.tensor_scalar(
                    out=FT[:, 1196:1197], in0=sm[0:OUTF, :], scalar1=0.0,
                    scalar2=None, op0=ALU.not_equal)
                nc.vector.tensor_scalar(
                    out=FT[:, 1197:1198], in0=sm[0:OUTF, :], scalar1=0.0,
                    scalar2=1.0, op0=ALU.not_equal, op1=ALU.add)

            nc.scalar.dma_start(yr[:], FEATR[:])
            nc.scalar.dma_start(yl[:], FEATL[:])

    nc.compile()
    return nc


_NC_CACHE = None


def _get_nc():
    global _NC_CACHE
    if _NC_CACHE is None:
        _NC_CACHE = build_bass()
    return _NC_CACHE


def make_in_maps(x: np.ndarray):
    x = np.asarray(x, dtype=np.float32)
    assert x.shape == (T_TOT, 115, 3)
    cst = _build_cst()
    in_maps = []
    for c in range(NCORES):
        s = c * SHARD
        xb = np.ascontiguousarray(x[c * OUTF:c * OUTF + BF])
        xlp = np.zeros((SHARD_PAD, 63), np.float32)
        xlp[:SHARD] = x[s:s + SHARD, 40:61, :].reshape(SHARD, 63)
        xrp = np.zeros((SHARD_PAD, 63), np.float32)
        xrp[:SHARD] = x[s:s + SHARD, 94:115, :].reshape(SHARD, 63)
        in_maps.append({
            "xl": xlp,
            "xr": xrp,
            "xb": xb,
            "xstk": _build_xstk(xb),
            "cst": cst,
        })
    return in_maps


def run_device(x: np.ndarray, **kw):
    nc = _get_nc()
    in_maps = make_in_maps(x)
    res = bass_utils.run_bass_kernel_spmd(
        nc, in_maps, core_ids=list(range(NCORES)), **kw)
    # global left/right decision from the exact integer-valued partials
    diff = sum(float(np.sum(np.float64(r["pdif"][:, 0])
                            - np.float64(r["pdif"][:, 1])))
               for r in res.results)
    key = "yl" if diff > 0 else "yr"
    out = np.concatenate([r[key] for r in res.results], axis=0)
    return out.reshape(1, 200, 1198).astype(np.float32, copy=False), res


def kernel(x: np.ndarray) -> np.ndarray:
    return run_device(x)[0]


if __name__ == "__main__":
    rng = np.random.default_rng(0)
    x = rng.standard_normal((T_TOT, 115, 3), dtype=np.float32)
    out = kernel(x)
    print(out.shape, out.dtype, float(np.linalg.norm(out)))


# revision 40
# speedup vs baseline: 1.1041x; 1.1041x over previous
"""Trainium2 Bass kernel for nn_FeatureGenKerasV2.

Contract: kernel(x) with x [100000, 115, 3] f32 -> [1, 200, 1198] f32.

Reference semantics:
  - global: cond = (count_nonzero(x[:,40:61]) > count_nonzero(x[:,94:115]))
  - per frame t<200: features built from hand(sel by cond)/pose/lip coords,
    temporal diff vs frame t+1, static-pair distances, hand mask.

Sharding (8 cores):
  - count phase: core c receives the two dense hand-column slices
    xl/xr = x[12500c:12500(c+1), 40:61|94:115, :] (column sharding of x),
    streams them at line rate and counts nonzeros (left on DVE, right on
    GpSimd), emitting per-partition partial counts.
  - feature phase: core c computes BOTH left/right feature variants for its
    output frames [25c, 25c+26) and writes yl_c/yr_c [25, 1198].
  - unshard: the host sums the per-core integer-valued partials, picks the
    variant (cond = cntL > cntR), and concatenates the per-core slices.
"""

import numpy as np

import concourse.bass as bass
import concourse.tile as tile
from concourse import bacc, mybir
from concourse import bass_utils

F32 = mybir.dt.float32
BF16 = mybir.dt.bfloat16
ALU = mybir.AluOpType

NCORES = 8
T_TOT = 100000
SHARD = T_TOT // NCORES          # 12500 count frames per core
P = 128                          # SBUF partitions used for counting; the
                                 # shard is zero-padded to 128*98 frames so
                                 # count DMAs have 128 descriptors (= 16
                                 # SDMA engines; 125 descs land on only 5)
FPP = 98                         # frames per partition (padded)
SHARD_PAD = P * FPP              # 12544
HW_ = FPP * 63                   # whole-hand row width per partition (6174)
CW = HW_ // 2                    # count chunk width (3087)
SW = CW // 2                     # tail sub-op width
OUTF = 25                        # output frames per core
BF = OUTF + 1                    # feature frames per core (1 halo)

# static pair index tables (match np.triu_indices order used by reference)
_HIU = np.triu_indices(21, 1)    # 210 hand pairs
_PIU = np.triu_indices(25, 1)    # 300 pose pairs
_LIU = np.triu_indices(20, 1)    # 190 lip pairs
NH, NP_, NL = 210, 300, 190

# packed constant layout (cst [65, 1406]):
#   [0:420)    GH  block-diag: rows 0:21 handL pairs, rows 21:42 handR pairs
#   [420:910)  G1  block-diag: rows 0:25 pose pairs, rows 25:45 outer-lip
#   [910:1100) G2  rows 45:65 inner-lip pairs
#   [1100:1406) M  rows 0:26: +-1 x-mirror mask over feat cols 0:306
CST_W = 1406
# packed per-core frame stacks (xstk [65, 130]):
#   [0:78)   hands,  rows 0:21 L / 21:42 R, col = coord*26 + frame
#   [78:130) shared, rows 0:25 pose / 25:45 lip1 / 45:65 lip2, 2 coords
XSTK_W = 130


def _pairs(g, iu, r0, c0):
    n = len(iu[0])
    g[r0 + iu[0], c0 + np.arange(n)] = 1.0
    g[r0 + iu[1], c0 + np.arange(n)] -= 1.0


def _build_cst():
    cst = np.zeros((65, CST_W), np.float32)
    _pairs(cst, _HIU, 0, 0)
    _pairs(cst, _HIU, 21, 210)
    _pairs(cst, _PIU, 0, 420)
    _pairs(cst, _LIU, 25, 720)
    _pairs(cst, _LIU, 45, 910)
    m = np.ones((26, 306), np.float32)
    m[:, 0:63:3] = -1.0      # hand x
    m[:, 63:153:2] = -1.0    # pose/lip x
    m[:, 153:216:3] = -1.0   # dhand x
    m[:, 216:306:2] = -1.0   # dpose/dlip x
    cst[0:26, 1100:1406] = m
    return cst


def _build_xstk(xb):
    # xb: [26, 115, 3]
    xstk = np.zeros((65, XSTK_W), np.float32)
    xstk[0:21, 0:78] = xb[:, 40:61, :].transpose(1, 2, 0).reshape(21, 78)
    xstk[21:42, 0:78] = xb[:, 94:115, :].transpose(1, 2, 0).reshape(21, 78)
    xstk[0:25, 78:130] = xb[:, 61:86, 0:2].transpose(1, 2, 0).reshape(25, 52)
    xstk[25:45, 78:130] = xb[:, 0:20, 0:2].transpose(1, 2, 0).reshape(20, 52)
    xstk[45:65, 78:130] = xb[:, 20:40, 0:2].transpose(1, 2, 0).reshape(20, 52)
    return xstk


def build_bass():
    nc = bacc.Bacc("TRN2", target_bir_lowering=False, debug=False,
                   num_devices=NCORES)

    xl = nc.dram_tensor("xl", [SHARD_PAD, 63], F32, kind="ExternalInput")
    xr = nc.dram_tensor("xr", [SHARD_PAD, 63], F32, kind="ExternalInput")
    xb = nc.dram_tensor("xb", [BF, 115, 3], F32, kind="ExternalInput")
    xstk = nc.dram_tensor("xstk", [65, XSTK_W], F32, kind="ExternalInput")
    cst = nc.dram_tensor("cst", [65, CST_W], F32, kind="ExternalInput")
    yl = nc.dram_tensor("yl", [OUTF, 1198], F32, kind="ExternalOutput")
    yr = nc.dram_tensor("yr", [OUTF, 1198], F32, kind="ExternalOutput")
    pdif = nc.dram_tensor("pdif", [P, 2], F32, kind="ExternalOutput")

    xlr = xl[:].rearrange("(p f) c -> p (f c)", p=P)   # [125, 6300]
    xrr = xr[:].rearrange("(p f) c -> p (f c)", p=P)

    with tile.TileContext(nc) as tc:
        with (
            tc.tile_pool(name="cl", bufs=2) as cl,
            tc.tile_pool(name="cr", bufs=2) as cr,
            tc.tile_pool(name="scr", bufs=2) as scr,
            tc.tile_pool(name="persist", bufs=1) as persist,
            tc.tile_pool(name="fb", bufs=1) as fb,
            tc.tile_pool(name="psum", bufs=1, space=bass.MemorySpace.PSUM) as psum,
        ):
            # ---------------- count phase ----------------
            # Two SWDGE chunks per hand (12.6KB descriptors, 16-engine
            # spread), streamed R0 L0 R1 L1; all counting is fused
            # not_equal+accumulate on DVE (CACHE_REDUCE). The final chunk's
            # compute is split into two column sub-ops to shorten the tail.
            accL = persist.tile([P, 3], F32)
            accR = persist.tile([P, 2], F32)
            # feature inputs load first on the sync ring (FIFO): they are
            # small (~0.5MB) and must not queue behind the count stream
            CST = fb.tile([65, CST_W], F32)
            nc.sync.dma_start(CST[:], cst[:])
            XSTK = fb.tile([65, XSTK_W], F32)
            nc.sync.dma_start(XSTK[:], xstk[:])
            XB = fb.tile([BF, 115, 3], F32)
            nc.sync.dma_start(XB[:], xb[:])
            XBs = fb.tile([OUTF, 115, 3], F32)
            nc.sync.dma_start(XBs[:], XB[1:BF, :, :])

            tiles = []
            for k in range(2):
                sl = slice(k * CW, (k + 1) * CW)
                tR = cr.tile([P, CW], F32, tag="cR", name="tR")
                nc.sync.dma_start(tR[:], xrr[:, sl])
                tL = cl.tile([P, CW], F32, tag="cL", name="tL")
                nc.sync.dma_start(tL[:], xlr[:, sl])
                tiles.append((tR, tL))

            def cache_count(src_ap, acc_col, w):
                s_ = scr.tile([P, w], BF16, tag=f"s{w}", name="s_")
                nc.vector.tensor_scalar(
                    out=s_[:], in0=src_ap, scalar1=0.0, scalar2=None,
                    op0=ALU.not_equal, op1=ALU.add, accum_out=acc_col)

            cache_count(tiles[0][0][:], accR[:, 0:1], CW)   # R0
            cache_count(tiles[0][1][:], accL[:, 0:1], CW)   # L0
            cache_count(tiles[1][0][:], accR[:, 1:2], CW)   # R1
            cache_count(tiles[1][1][:, 0:SW], accL[:, 1:2], SW)   # L1a
            cache_count(tiles[1][1][:, SW:CW], accL[:, 2:3], CW - SW)  # L1b

            red = persist.tile([P, 2], F32)
            nc.vector.reduce_sum(out=red[:, 0:1], in_=accL[:],
                                 axis=mybir.AxisListType.X)
            nc.vector.reduce_sum(out=red[:, 1:2], in_=accR[:],
                                 axis=mybir.AxisListType.X)
            nc.scalar.dma_start(pdif[:], red[:])

            # ---------------- feature phase ----------------
            D = fb.tile([OUTF, 115, 3], F32)
            nc.gpsimd.tensor_sub(D[:], XB[0:OUTF, :, :], XBs[:])

            # pairwise coordinate diffs via 7 block-diagonal matmuls
            ps = []
            for c in range(3):          # hands, coords x/y/z -> [26, 420]
                ph = psum.tile([BF, 420], F32, tag=f"ph{c}", name="ph")
                nc.tensor.matmul(ph[:], XSTK[0:42, c * BF:(c + 1) * BF],
                                 CST[0:42, 0:420])
                ps.append(ph)
            pa = []
            for c in range(2):          # pose+outer lip -> [26, 490]
                p_ = psum.tile([BF, 490], F32, tag=f"pa{c}", name="pa")
                nc.tensor.matmul(p_[:], XSTK[0:45, 78 + c * BF:78 + (c + 1) * BF],
                                 CST[0:45, 420:910])
                pa.append(p_)
            pb = []
            for c in range(2):          # inner lip -> [26, 190]
                p_ = psum.tile([BF, 190], F32, tag=f"pb{c}", name="pb")
                nc.tensor.matmul(p_[:], XSTK[0:65, 78 + c * BF:78 + (c + 1) * BF],
                                 CST[0:65, 910:1100])
                pb.append(p_)

            # dsq layout: [hL 210 | hR 210 | pose 300 | olip 190 | ilip 190]
            dsq = fb.tile([BF, 1100], F32)
            sy = fb.tile([BF, 1100], F32)
            sz = fb.tile([BF, 420], F32)
            nc.scalar.square(dsq[:, 0:420], ps[0][:])
            nc.scalar.square(sy[:, 0:420], ps[1][:])
            nc.scalar.square(sz[:], ps[2][:])
            nc.scalar.square(dsq[:, 420:910], pa[0][:])
            nc.scalar.square(sy[:, 420:910], pa[1][:])
            nc.scalar.square(dsq[:, 910:1100], pb[0][:])
            nc.scalar.square(sy[:, 910:1100], pb[1][:])
            nc.gpsimd.tensor_add(dsq[:], dsq[:], sy[:])
            nc.gpsimd.tensor_add(dsq[:, 0:420], dsq[:, 0:420], sz[:])

            FEATL = fb.tile([OUTF, 1198], F32)
            FEATR = fb.tile([OUTF, 1198], F32)

            def v3(ft, lo, hi):
                return ft[:, lo:hi].rearrange("p (j c) -> p j c", c=3)

            def v2(ft, lo, hi):
                return ft[:, lo:hi].rearrange("p (j c) -> p j c", c=2)

            # right variant coordinate blocks
            nc.scalar.copy(v3(FEATR, 0, 63), XB[0:OUTF, 94:115, :])
            nc.scalar.copy(v2(FEATR, 63, 113), XB[0:OUTF, 61:86, 0:2])
            nc.scalar.copy(v2(FEATR, 113, 153), XB[0:OUTF, 0:20, 0:2])
            nc.scalar.copy(v3(FEATR, 153, 216), D[:, 94:115, :])
            nc.scalar.copy(v2(FEATR, 216, 266), D[:, 61:86, 0:2])
            nc.scalar.copy(v2(FEATR, 266, 306), D[:, 0:20, 0:2])
            # left variant: own hand blocks, shared pose/lip copied from R,
            # then a single +-1 mirror multiply over cols 0:306
            nc.scalar.copy(v3(FEATL, 0, 63), XB[0:OUTF, 40:61, :])
            nc.scalar.copy(v3(FEATL, 153, 216), D[:, 40:61, :])
            nc.scalar.copy(FEATL[:, 63:153], FEATR[:, 63:153])
            nc.scalar.copy(FEATL[:, 216:306], FEATR[:, 216:306])
            nc.gpsimd.tensor_mul(FEATL[:, 0:306], FEATL[:, 0:306],
                                 CST[0:OUTF, 1100:1406])

            # distances
            nc.scalar.sqrt(FEATR[:, 306:1196], dsq[0:OUTF, 210:1100])
            nc.scalar.sqrt(FEATL[:, 306:516], dsq[0:OUTF, 0:210])
            nc.gpsimd.tensor_copy(FEATL[:, 516:1196], FEATR[:, 516:1196])

            # hand masks + token type ids
            sumL = fb.tile([BF, 1], F32)
            nc.vector.reduce_sum(out=sumL[:], in_=XB[:, 40:61, :],
                                 axis=mybir.AxisListType.XY)
            sumR = fb.tile([BF, 1], F32)
            nc.vector.reduce_sum(out=sumR[:], in_=XB[:, 94:115, :],
                                 axis=mybir.AxisListType.XY)
            for FT, sm in ((FEATL, sumL), (FEATR, sumR)):
                nc.vector.tensor_scalar(
                    out=FT[:, 1196:1197], in0=sm[0:OUTF, :], scalar1=0.0,
                    scalar2=None, op0=ALU.not_equal)
                nc.vector.tensor_scalar(
                    out=FT[:, 1197:1198], in0=sm[0:OUTF, :], scalar1=0.0,
                    scalar2=1.0, op0=ALU.not_equal, op1=ALU.add)

            nc.scalar.dma_start(yr[:], FEATR[:])
            nc.scalar.dma_start(yl[:], FEATL[:])

    nc.compile()
    return nc


_NC_CACHE = None


def _get_nc():
    global _NC_CACHE
    if _NC_CACHE is None:
        _NC_CACHE = build_bass()
    return _NC_CACHE


def make_in_maps(x: np.ndarray):
    x = np.asarray(x, dtype=np.float32)
    assert x.shape == (T_TOT, 115, 3)
    cst = _build_cst()
    in_maps = []
    for c in range(NCORES):
        s = c * SHARD
        xb = np.ascontiguousarray(x[c * OUTF:c * OUTF + BF])
        xlp = np.zeros((SHARD_PAD, 63), np.float32)
        xlp[:SHARD] = x[s:s + SHARD, 40:61, :].reshape(SHARD, 63)
        xrp = np.zeros((SHARD_PAD, 63), np.float32)
        xrp[:SHARD] = x[s:s + SHARD, 94:115, :].reshape(SHARD, 63)
        in_maps.append({
            "xl": xlp,
            "xr": xrp,
            "xb": xb,
            "xstk": _build_xstk(xb),
            "cst": cst,
        })
    return in_maps


def run_device(x: np.ndarray, **kw):
    nc = _get_nc()
    in_maps = make_in_maps(x)
    res = bass_utils.run_bass_kernel_spmd(
        nc, in_maps, core_ids=list(range(NCORES)), **kw)
    # global left/right decision from the exact integer-valued partials
    diff = sum(float(np.sum(np.float64(r["pdif"][:, 0])
                            - np.float64(r["pdif"][:, 1])))
               for r in res.results)
    key = "yl" if diff > 0 else "yr"
    out = np.concatenate([r[key] for r in res.results], axis=0)
    return out.reshape(1, 200, 1198).astype(np.float32, copy=False), res


def kernel(x: np.ndarray) -> np.ndarray:
    return run_device(x)[0]


if __name__ == "__main__":
    rng = np.random.default_rng(0)
    x = rng.standard_normal((T_TOT, 115, 3), dtype=np.float32)
    out = kernel(x)
    print(out.shape, out.dtype, float(np.linalg.norm(out)))
